# revision 37
# baseline (speedup 1.0000x reference)
"""LongNet dilated-attention fused Bass/Tile kernel for TRN2 (one core's program).

Math (per core, T tokens of the even-subsampled sequence, model dim D, H heads
of dim 64, NB branches with block sizes blocks[b]):

  for each branch b:
    Q = x W_q^T ; K = x W_k^T ; V = x W_v^T          (+ biases)
    block-diagonal attention with block m_b, softmax over k
    o_b = attn @ V
  y = sum_b o_b W_o[b]^T + biases

Device layout choices:
  - x is provided TRANSPOSED as xt [D, T] bf16 (host prep).
  - Weights provided transposed: wq [D, 3*D*NB] bf16 (per branch: Q|K|V
    column sections), wo [D*NB, D] bf16.
  - Q^T/K^T are produced per HEAD-PAIR chunk [P, 2, TG] (feature-major), not
    per branch: chunk hp holds Q features [hp*128,(hp+1)*128) on partitions
    (heads 2hp, 2hp+1) and the matching K features. The chunk for head-pair
    hp+1 is emitted as FILLER between the exp-gated attention quanta of
    head-pair hp, so the PE queue never stalls while the ACT engine computes
    exp, and the PE stays busy enough that HAM never throttles it to 1.2 GHz.
  - V GEMM produces V token-major [tok, feat] with a ones-column appended per
    head (65 cols per head) so the AV matmul also produces the softmax
    denominator row.
  - scores^T tiles [k, q] via lhsT=K^T, rhs=Q^T (K=64 contraction; the two
    heads of a pair use partition bases 0/64 -> concurrent PE row groups).
  - exp on ACT (scale folded), pairs of k-chunks share one 2-bank PSUM tile so
    each ACTIVATE covers 2*qsw columns.
  - AV: lhsT = [V_h | ones] [ktok, 65] -> psum [65, q]: rows 0..63 = o^T
    unnormalized, row 64 = denominator s.
  - normalize: rs = 1/s (DVE), broadcast across partitions (DRAM bounce +
    partition-broadcast DMA), fused into a DVE multiply on the o^T buffer.
  - out-projection: lhsT = o^T chunks, rhs = wo tiles, accumulate NB*D/128
    k-chunks in PSUM; eviction adds the (host-folded) output bias row.
    Runs dense at the end of each group (no exp pressure there).

Biases: Q/K biases are added exactly during QK eviction (per-partition scalar
add). V bias and output bias are folded on the host into the final bias row
(exact: attn rows sum to 1, so o = A(V + 1 b_v^T) = A V + 1 b_v^T).
"""

import os as _os
import sys as _sys
for _p in ("/opt/trn_rl_repo",):
    if _os.path.isdir(_p) and _p not in _sys.path:
        _sys.path.insert(0, _p)


from collections import deque
from contextlib import ExitStack
from dataclasses import dataclass

import numpy as np

import concourse.bass as bass
import concourse.mybir as mybir
import concourse.tile as tile

F32 = mybir.dt.float32
BF16 = mybir.dt.bfloat16
P = 128


@dataclass(frozen=True)
class Cfg:
    D: int = 1024
    H: int = 16
    T: int = 2048            # tokens per core
    TG: int = 1024           # token group (attention/out-proj granularity)
    blocks: tuple = (256, 512, 1024)

    @property
    def HD(self):
        return self.D // self.H

    @property
    def NB(self):
        return len(self.blocks)

    @property
    def dc_n(self):
        return self.D // P   # input-feature chunks

    @property
    def nfc(self):
        return self.D // P   # feature chunks per Q (or K) section

    @property
    def scale(self):
        return 1.0 / float(np.sqrt(np.float32(self.HD)))


def build(cfg: Cfg) -> bass.Bass:
    D, H, T, TG = cfg.D, cfg.H, cfg.T, cfg.TG
    NB, dc_n, nfc = cfg.NB, cfg.dc_n, cfg.nfc
    assert cfg.HD == 64, "head pairing assumes head dim 64"
    assert T % TG == 0 and TG % max(cfg.blocks) == 0
    assert min(cfg.blocks) >= P
    ntk = TG // P            # 128-token chunks per group
    tgp = TG // P
    TW = min(512, TG)        # QK gemm token slice
    WV = min(512, D)         # V gemm feature slice
    hv = WV // 64            # heads per V slice
    ndw = min(512, D)        # out-proj N slice
    nnd = D // ndw
    nkc_out = NB * nfc       # out-proj contraction chunks
    ngr = T // TG
    nhp = H // 2             # head pairs == feature chunks per section

    nc = bass.Bass(trn_type="TRN2", target_bir_lowering=False)

    xt = nc.dram_tensor("xt", [D, T], BF16, kind="ExternalInput")
    wq = nc.dram_tensor("wq", [D, 3 * D * NB], BF16, kind="ExternalInput")
    wo = nc.dram_tensor("wo", [D * NB, D], BF16, kind="ExternalInput")
    qb = nc.dram_tensor("qb", [P, 3 * nfc * NB], F32, kind="ExternalInput")
    yb = nc.dram_tensor("yb", [1, D], F32, kind="ExternalInput")
    y = nc.dram_tensor("y", [T, D], F32, kind="ExternalOutput")

    with tile.TileContext(nc) as tc, ExitStack() as ctx:
        const = ctx.enter_context(tc.tile_pool(name="const", bufs=1))
        xtp = ctx.enter_context(tc.tile_pool(name="xtp", bufs=2))
        qkp = ctx.enter_context(tc.tile_pool(name="qkp", bufs=3))
        vsp = ctx.enter_context(tc.tile_pool(name="vsp", bufs=1))
        osp = ctx.enter_context(tc.tile_pool(name="osp", bufs=1))
        wqp = ctx.enter_context(tc.tile_pool(name="wqp", bufs=3))
        wvp = ctx.enter_context(tc.tile_pool(name="wvp", bufs=1))
        wop = ctx.enter_context(tc.tile_pool(name="wop", bufs=nkc_out + nkc_out // 2))
        etp = ctx.enter_context(tc.tile_pool(name="etp", bufs=2))
        salp = ctx.enter_context(tc.tile_pool(name="salp", bufs=1))
        stp = ctx.enter_context(tc.tile_pool(name="stp", bufs=2))
        bcp = ctx.enter_context(tc.tile_pool(name="bcp", bufs=2))
        yp = ctx.enter_context(tc.tile_pool(name="yp", bufs=2))
        drp = ctx.enter_context(tc.tile_pool(name="drp", bufs=4, space="DRAM"))
        psS = ctx.enter_context(tc.tile_pool(name="psS", bufs=2, space="PSUM"))
        psA = ctx.enter_context(tc.tile_pool(name="psA", bufs=2, space="PSUM"))
        psG = ctx.enter_context(tc.tile_pool(name="psG", bufs=2, space="PSUM"))

        # ---- constants ----
        qb_s = const.tile([P, 3 * nfc * NB], F32, tag="qb")
        nc.sync.dma_start(qb_s[:], qb.ap())
        yb_bc = const.tile([P, D], F32, tag="ybbc")
        nc.sync.dma_start(yb_bc[:], yb.ap()[0, :].partition_broadcast(P))

        # V buffer with a ones column per head (written once; V evictions
        # never touch them): the AV matmul's 65th row is the softmax
        # denominator.
        VW = 65
        vs = vsp.tile([P, ntk, H * VW], BF16, tag="vs")
        for h in range(H):
            nc.gpsimd.memset(vs[:, :, h * VW + 64:h * VW + VW], 1.0)

        xtg_t = {}

        def load_xtg(g, split=False):
            xtg = xtp.tile([P, dc_n, TG], BF16, tag="xtg", name="xtg")
            if split:  # per-chunk DMAs so the first GEMM starts sooner
                for c in range(dc_n):
                    nc.sync.dma_start(
                        xtg[:, c, :], xt.ap()[c * P:(c + 1) * P, g * TG:(g + 1) * TG])
            else:
                nc.sync.dma_start(
                    xtg[:],
                    xt.ap()[:, g * TG:(g + 1) * TG].rearrange("(c p) t -> p c t", p=P),
                )
            xtg_t[g] = xtg

        # o^T buffers, split into (branches 0,1) / (branch 2) so the
        # out-projection's first contraction chunks don't wait on the last
        # branch's normalization (dependencies on these tiles are coarse).
        os_t = {}

        def alloc_os(g):
            os_t[g] = (
                osp.tile([P, 2 * nfc, TG], BF16, tag="osa", name="osa"),
                osp.tile([P, nfc, TG], BF16, tag="osb", name="osb"),
            )

        def os_slice(g, oc):
            if oc < 2 * nfc:
                return os_t[g][0][:, oc, :]
            return os_t[g][1][:, oc - 2 * nfc, :]

        # ------------------------------------------------------------------
        # filler machinery: thunks that emit ACT-independent tensor work
        # ------------------------------------------------------------------
        filler = deque()

        def fill(n):
            for _ in range(min(n, len(filler))):
                filler.popleft()()

        def drain():
            fill(len(filler))

        # ---- QK head-pair chunk: qk tile [P, 2(section Q|K), TG] ----
        qk_tiles = {}

        def push_qk_chunk(g, b, hp):
            """Append 16 half-group thunks producing qk chunk (g, b, hp)."""
            base = b * 3 * D
            st = {}

            def ensure():
                if "qk" in st:
                    return
                st["qk"] = qkp.tile([P, 2, TG], BF16, tag="qk", name="qk")
                qk_tiles[(g, b, hp)] = st["qk"]
                for qki in (0, 1):
                    wqt = wqp.tile([P, dc_n, P], BF16, tag="wqt", name="wqt")
                    nc.sync.dma_start(
                        wqt[:],
                        wq.ap()[:, base + qki * D + hp * P:
                                base + qki * D + (hp + 1) * P]
                        .rearrange("(c p) f -> p c f", p=P),
                    )
                    st[f"w{qki}"] = wqt

            def mk(qki, t2, half):
                def th():
                    ensure()
                    xtg = xtg_t[g]
                    wqt = st[f"w{qki}"]
                    if half == 0:
                        st["ps"] = psG.tile([P, 512], F32, tag="g", name="psq")
                    ps = st["ps"]
                    for dc in range(half * (dc_n // 2), (half + 1) * (dc_n // 2)):
                        nc.tensor.matmul(
                            ps[:, :TW],
                            wqt[:, dc, :],
                            xtg[:, dc, t2 * TW:(t2 + 1) * TW],
                            start=dc == 0,
                            stop=dc == dc_n - 1,
                        )
                    if half == 1:
                        col = b * 3 * nfc + qki * nfc + hp
                        nc.vector.tensor_scalar_add(
                            st["qk"][:, qki, t2 * TW:(t2 + 1) * TW],
                            ps[:, :TW],
                            qb_s[:, col:col + 1],
                        )
                return th

            for qki in (0, 1):
                for t2 in range(TG // TW):
                    filler.append(mk(qki, t2, 0))
                    filler.append(mk(qki, t2, 1))

        # ---- V gemm slice vf (heads vf*hv .. vf*hv+hv-1) ----
        def push_v_slice(g, b, vf):
            base = b * 3 * D + 2 * D
            st = {}

            def ensure():
                if "wv" in st:
                    return
                wv = wvp.tile([P, dc_n, WV], BF16, tag="wv", name="wv")
                nc.sync.dma_start(
                    wv[:],
                    wq.ap()[:, base + vf * WV: base + (vf + 1) * WV]
                    .rearrange("(c p) f -> p c f", p=P),
                )
                st["wv"] = wv

            def mk(tk, half):
                def th():
                    ensure()
                    xtg = xtg_t[g]
                    if half == 0:
                        st["ps"] = psG.tile([P, 512], F32, tag="g", name="psv")
                    ps = st["ps"]
                    for dc in range(half * (dc_n // 2), (half + 1) * (dc_n // 2)):
                        nc.tensor.matmul(
                            ps[:, :WV],
                            xtg[:, dc, tk * P:(tk + 1) * P],
                            st["wv"][:, dc, :],
                            start=dc == 0,
                            stop=dc == dc_n - 1,
                        )
                    if half == 1:
                        nc.vector.tensor_copy(
                            vs[:, tk, vf * hv * VW:(vf * hv + hv) * VW]
                            .rearrange("p (h x) -> p h x", x=VW)[:, :, 0:64],
                            ps[:, :WV].rearrange("p (h f) -> p h f", f=64),
                        )
                return th

            for tk in range(ntk):
                filler.append(mk(tk, 0))
                filler.append(mk(tk, 1))

        # ---- attention unit: one (branch, head-pair, block), all q-slices ----
        # The two heads of a pair share ONE psum tile per k-chunk (head 0 in
        # bank A columns, head 1 in bank B) evicted by ONE exp ACTIVATE, so
        # both scores matmuls wait on the same event and issue concurrently
        # (disjoint PE row groups). Q-slices loop inside the k-chunk loop so
        # consecutive pairs reuse the loaded K weights.
        def emit_unit(g, b, oc_base, hp, bl, f_sc, f_av):
            m = cfg.blocks[b]
            kcw = P
            kc_n = m // kcw
            qsw = min(512, m)
            nqs = m // qsw
            kt0 = bl * m
            qk = qk_tiles[(g, b, hp)]
            ets = [etp.tile([P, 2, kc_n, qsw], BF16, tag="et", name=f"ets{qs}")
                   for qs in range(nqs)]
            for kc in range(kc_n):
                for qs in range(nqs):
                    q0 = kt0 + qs * qsw
                    pss = psS.tile([P, 1024], F32, tag="s", name="pss")
                    for hh in (0, 1):
                        nc.tensor.matmul(
                            pss[:kcw, hh * 512: hh * 512 + qsw],
                            qk[hh * 64:hh * 64 + 64, 1,
                               kt0 + kc * kcw: kt0 + (kc + 1) * kcw],
                            qk[hh * 64:hh * 64 + 64, 0, q0:q0 + qsw],
                            start=True,
                            stop=True,
                        )
                    nc.scalar.activation(
                        ets[qs][:kcw, :, kc, :],
                        pss[:kcw, :].rearrange("p (a q) -> p a q", a=2)[:, :, :qsw],
                        mybir.ActivationFunctionType.Exp,
                        scale=cfg.scale,
                    )
                if kc % 2 == 1:
                    fill(f_sc)
            # AV + denominator rows (replicated x4 by the ones columns)
            for hh in (0, 1):
                h = 2 * hp + hh
                for qs in range(nqs):
                    q0 = kt0 + qs * qsw
                    pso = psA.tile([VW, 512], F32, tag="a", name="pso")
                    for kc in range(kc_n):
                        tok = kt0 + kc * kcw
                        tkc = tok // P
                        nc.tensor.matmul(
                            pso[:, :qsw],
                            vs[0:kcw, tkc, h * VW:(h + 1) * VW],
                            ets[qs][:kcw, hh, kc, :],
                            start=kc == 0,
                            stop=kc == kc_n - 1,
                        )
                    # split eviction work: small-block branches are DVE-queue
                    # bound, so head 0 of the pair evicts via the ACT engine
                    ev = nc.scalar if (hh == 0 and kc_n == 2) else nc.vector
                    if ev is nc.scalar:
                        ev.copy(
                            os_slice(g, oc_base + hp)[hh * 64:hh * 64 + 64,
                                                      q0:q0 + qsw],
                            pso[0:64, :qsw],
                        )
                    else:
                        ev.tensor_copy(
                            os_slice(g, oc_base + hp)[hh * 64:hh * 64 + 64,
                                                      q0:q0 + qsw],
                            pso[0:64, :qsw],
                        )
                    stg = stp.tile([1, 512], BF16, tag="stg", name="stg")
                    with nc.allow_low_precision(reason="softmax denom in bf16"):
                        if ev is nc.scalar:
                            ev.copy(stg[:, :qsw], pso[64:65, :qsw])
                        else:
                            ev.tensor_copy(stg[:, :qsw], pso[64:65, :qsw])
                    (nc.sync if hh == 0 else nc.gpsimd).dma_start(
                        sal_t[h * tgp + q0 // P: h * tgp + q0 // P + qsw // P, :],
                        stg[:1, :qsw],
                    )
            fill(f_av)

        # ---- normalization (per branch, heads [h0, h1)) ----
        def emit_norm(g, oc_base, h0, h1):
            r0, r1 = h0 * tgp, h1 * tgp
            with nc.allow_low_precision(reason="1/s row in bf16"):
                nc.vector.reciprocal(salr_t[r0:r1, :], sal_t[r0:r1, :])
            nc.gpsimd.dma_start(scr2_t[r0:r1, :], salr_t[r0:r1, :])
            for hp in range(h0 // 2, h1 // 2):
                bc = bcp.tile([P, TG], BF16, tag="bc", name="bc")
                for hh in (0, 1):
                    h = 2 * hp + hh
                    eng = nc.sync if hh == 0 else nc.gpsimd
                    eng.dma_start(
                        bc[hh * 64:(hh + 1) * 64, :]
                        .rearrange("p (a c) -> p a c", c=P),
                        scr2_t[h * tgp:(h + 1) * tgp, :]
                        .partition_broadcast(64))
                oc = oc_base + hp
                nc.vector.tensor_tensor(
                    os_slice(g, oc), os_slice(g, oc), bc[:],
                    mybir.AluOpType.mult)

        # ---- partial out-proj chains (branches 0,1 contraction only) ----
        # Used as tail filler for the very last head-pair: these 16-MM chain
        # prefixes depend only on the already-normalized os_a buffer. The
        # psum tiles stay open until emit_out finishes the chains.
        out_partial = {}

        def push_out_partial(g, wots, tks):
            def mk(tk, quarter):
                def th():
                    if quarter == 0:
                        out_partial[(g, 0, tk)] = psG.tile(
                            [P, 512], F32, tag="g", name="psy")
                    psy = out_partial[(g, 0, tk)]
                    for kc in range(quarter * 4, (quarter + 1) * 4):
                        nc.tensor.matmul(
                            psy[:, :ndw],
                            os_slice(g, kc)[:, tk * P:(tk + 1) * P],
                            wots[kc][:],
                            start=kc == 0,
                            stop=False,
                        )
                return th
            for tk in tks:
                for quarter in range(4):
                    filler.append(mk(tk, quarter))

        # out-proj contraction index kc -> wo row block, following the branch
        # processing order (so the last-processed branch's chunks come last)
        def wo_row(kc):
            return PROC[kc // nfc] * D + (kc % nfc) * P

        # ---- out-projection for group g (dense) ----
        def emit_out(g, wots0, wots1_early):
            for nd in range(nnd):
                if nd == 0:
                    wots = wots0
                else:
                    # first half was prefetched into fresh pool buffers at
                    # branch start; the rest reuses nd0's buffers and must be
                    # issued here (a WAR-blocked DMA at the queue head would
                    # stall every later DMA on this queue).
                    wots = list(wots1_early)
                    for kc in range(nkc_out // 2, nkc_out):
                        wt = wop.tile([P, ndw], BF16, tag="wot", name="wt")
                        nc.gpsimd.dma_start(
                            wt[:],
                            wo.ap()[wo_row(kc):wo_row(kc) + P,
                                    nd * ndw:(nd + 1) * ndw],
                        )
                        wots.append(wt)
                for tk in range(ntk):
                    psy = out_partial.pop((g, nd, tk), None)
                    kc0 = 0 if psy is None else 2 * nfc
                    if psy is None:
                        psy = psG.tile([P, 512], F32, tag="g", name="psy")
                    for kc in range(kc0, nkc_out):
                        nc.tensor.matmul(
                            psy[:, :ndw],
                            os_slice(g, kc)[:, tk * P:(tk + 1) * P],
                            wots[kc][:],
                            start=kc == 0,
                            stop=kc == nkc_out - 1,
                        )
                    ys = yp.tile([P, 512], F32, tag="ys", name="ys")
                    nc.vector.tensor_tensor(
                        ys[:, :ndw],
                        psy[:, :ndw],
                        yb_bc[:, nd * ndw:(nd + 1) * ndw],
                        mybir.AluOpType.add,
                    )
                    nc.gpsimd.dma_start(
                        y.ap()[g * TG + tk * P: g * TG + (tk + 1) * P,
                               nd * ndw:(nd + 1) * ndw],
                        ys[:, :ndw],
                    )

        # ------------------------------------------------------------------
        # main schedule
        # ------------------------------------------------------------------
        # filler pops per quantum (half-groups of 4 MMs), tuned so one QK
        # chunk (16 halves) spreads over one head-pair's units: blocks
        # (256,512,1024) have (4,2,1) units/hp and (1,2,4) fill points/unit.
        F_SC = {0: 2, 1: 2, 2: 3}
        F_AV = {0: 2, 1: 2, 2: 4}

        PROC = (2, 1, 0)   # branch processing order: end on the small branch
        seq = [(g, b) for g in range(ngr) for b in PROC]
        load_xtg(0, split=True)
        alloc_os(0)
        push_qk_chunk(0, PROC[0], 0)
        drain()

        for gi, (g, b) in enumerate(seq):
            m = cfg.blocks[b]
            nbl = TG // m
            kc_n = m // P
            oc_base = PROC.index(b) * nfc

            # per-branch denominator staging
            sal_t = salp.tile([H * tgp, P], BF16, tag="sal", name="sal")
            salr_t = salp.tile([H * tgp, P], BF16, tag="salr", name="salr")
            scr2_t = drp.tile([H * tgp, P], BF16, tag="scr2", name="scr2")

            drain()
            # V slice 0 becomes filler for the first unit of the branch so
            # its exps pipeline under the V GEMM instead of stalling the AV.
            push_v_slice(g, b, 0)
            f_first = 16 // max(1, kc_n // 2)

            if b == PROC[0]:
                # prefetch for the next group
                if g + 1 < ngr:
                    load_xtg(g + 1)
            if b == PROC[-1]:
                wots0 = []
                for kc in range(nkc_out):
                    wt = wop.tile([P, ndw], BF16, tag="wot", name="wt0")
                    nc.gpsimd.dma_start(
                        wt[:], wo.ap()[wo_row(kc):wo_row(kc) + P, 0:ndw])
                    wots0.append(wt)
                wots1_early = []

            last_branch = gi == len(seq) - 1
            for hp in range(nhp):
                if hp > 0:
                    drain()
                if hp + 1 < nhp:
                    push_qk_chunk(g, b, hp + 1)
                elif gi + 1 < len(seq):
                    g2, b2 = seq[gi + 1]
                    if g2 != g:
                        alloc_os(g2)
                    push_qk_chunk(g2, b2, 0)
                elif last_branch:
                    # tail filler: out-proj chain prefixes over branches 0,1
                    push_out_partial(g, wots0, (0, 1))
                if hp == 2 and D > WV:
                    push_v_slice(g, b, 1)
                if b == PROC[-1] and hp == 4:
                    for kc in range(nkc_out // 2):
                        wt = wop.tile([P, ndw], BF16, tag="wot", name="wt1")
                        (nc.sync if kc % 2 else nc.gpsimd).dma_start(
                            wt[:],
                            wo.ap()[wo_row(kc):wo_row(kc) + P, ndw:2 * ndw])
                        wots1_early.append(wt)
                for bl in range(nbl):
                    first = hp == 0 and bl == 0
                    emit_unit(g, b, oc_base, hp, bl,
                              f_first if first else F_SC[b],
                              F_AV[b])
                if hp == nhp // 2 - 1:
                    emit_norm(g, oc_base, 0, H // 2)
                elif last_branch and hp == 3 * nhp // 4 - 1:
                    emit_norm(g, oc_base, H // 2, 3 * H // 4)
            if last_branch:
                emit_norm(g, oc_base, 3 * H // 4, H)
            else:
                emit_norm(g, oc_base, H // 2, H)

            if b == PROC[-1]:
                drain()
                emit_out(g, wots0, wots1_early)

        drain()

    return nc


# ---------------- host-side helpers ----------------

def host_prep(cfg: Cfg, weights: dict) -> dict:
    """Build the per-core replicated input tensors from raw nn.Module weights.

    weights: {qkv_w{i}, qkv_b{i}, out_w{i}, out_b{i}} numpy arrays.
    Returns dict of numpy arrays keyed by dram tensor name (minus xt).
    """
    import ml_dtypes

    D, NB, nfc = cfg.D, cfg.NB, cfg.nfc
    bf16 = ml_dtypes.bfloat16
    wq = np.concatenate(
        [np.ascontiguousarray(weights[f"qkv_w{i}"].T) for i in range(NB)], axis=1
    ).astype(bf16)                                   # [D, 3D*NB]
    wo = np.concatenate(
        [np.ascontiguousarray(weights[f"out_w{i}"].T) for i in range(NB)], axis=0
    ).astype(bf16)                                   # [D*NB, D]
    qb = np.zeros((P, 3 * nfc * NB), np.float32)
    for i in range(NB):
        qb[:, i * 3 * nfc:(i + 1) * 3 * nfc] = (
            weights[f"qkv_b{i}"].astype(np.float32).reshape(3 * nfc, P).T
        )
    ybv = np.zeros((D,), np.float64)
    for i in range(NB):
        ybv += weights[f"out_b{i}"].astype(np.float64)
        ybv += weights[f"qkv_b{i}"][2 * D:3 * D].astype(np.float64) @ weights[
            f"out_w{i}"].astype(np.float64).T
    yb = ybv.astype(np.float32).reshape(1, D)
    return {"wq": wq, "wo": wo, "qb": qb, "yb": yb}


# ---------------- harness-facing entry point ----------------
# Shapes hardcoded per the contest contract: x (4, 8192, 1024) fp32, three
# branches of qkv/out weights. All three LongNet branches use rate=2 with
# even segment sizes, so they all read the same even tokens x[:, ::2, :] and
# differ only in attention block size (256/512/1024). The 16384 even tokens
# are split into 8 contiguous shards of 2048 (a multiple of the largest
# block): pure data parallelism, weights replicated, no collectives.

import ml_dtypes
from concourse.bass_utils import run_bass_kernel_spmd

_CFG = Cfg()  # D=1024, H=16, T=2048, TG=1024, blocks=(256, 512, 1024)
N_CORES = 8
B, S = 4, 8192

_NC_CACHE = None


def _split_sync_waits(nc, max_waits=1):
    """This neuronxcc build accepts at most one sync-wait per instruction;
    hoist extras onto their own EventSemaphore instructions (same engine --
    engine waits serialize, so semantics are unchanged)."""
    n = 0
    for f in nc.m.functions:
        for bb in f.blocks:
            out, changed = [], False
            for inst in bb.instructions:
                si = inst.sync_info
                if si is not None and si.on_wait and len(si.on_wait) > max_waits:
                    waits = list(si.on_wait)
                    for w in waits[:-max_waits]:
                        n += 1
                        out.append(mybir.InstEventSemaphore(
                            name=f"I-waitsplit-{n}",
                            engine=inst.engine,
                            sync_info=mybir.SyncInfo(on_wait=[w], on_update=[]),
                        ))
                    inst.sync_info = mybir.SyncInfo(
                        on_wait=waits[-max_waits:], on_update=list(si.on_update))
                    changed = True
                out.append(inst)
            if changed:
                bb.instructions.clear()
                bb.instructions.extend(out)
    return n


def get_nc():
    global _NC_CACHE
    if _NC_CACHE is None:
        nc = build(_CFG)
        _split_sync_waits(nc)
        _NC_CACHE = nc
    return _NC_CACHE


def make_in_maps(inputs):
    x = np.asarray(inputs["x"])
    xe = np.ascontiguousarray(x[:, ::2, :]).reshape(N_CORES, _CFG.T, _CFG.D)
    common = host_prep(_CFG, inputs)
    maps = []
    for c in range(N_CORES):
        mp = dict(common)
        mp["xt"] = np.ascontiguousarray(xe[c].T).astype(ml_dtypes.bfloat16)
        maps.append(mp)
    return maps


def kernel(x, qkv_w0, qkv_b0, out_w0, out_b0,
           qkv_w1, qkv_b1, out_w1, out_b1,
           qkv_w2, qkv_b2, out_w2, out_b2):
    inputs = dict(x=x, qkv_w0=qkv_w0, qkv_b0=qkv_b0, out_w0=out_w0,
                  out_b0=out_b0, qkv_w1=qkv_w1, qkv_b1=qkv_b1, out_w1=out_w1,
                  out_b1=out_b1, qkv_w2=qkv_w2, qkv_b2=qkv_b2, out_w2=out_w2,
                  out_b2=out_b2)
    nc = get_nc()
    in_maps = make_in_maps(inputs)
    res = run_bass_kernel_spmd(nc, in_maps, list(range(N_CORES)))
    yout = np.concatenate([res.results[c]["y"] for c in range(N_CORES)], axis=0)
    return yout.reshape(B, S // 2, _CFG.D)
